# revision 5
# baseline (speedup 1.0000x reference)
"""Trainium2 Bass kernel for ChronosMOEFeedForward (8-expert top-2 MoE + shared expert).

Strategy (expert-parallel over 8 NeuronCores):
  - Host computes the (tiny) gate: softmax(x @ gate_w.T), top-2, normalized
    combine weights; dispatches each token to its 2 experts.
  - Core e runs a SwiGLU FFN over the tokens routed to expert e (gathered,
    transposed, padded to capacity C), scaling rows by the combine weight
    during PSUM eviction. Each core also processes a 256-token slice of the
    shared expert (data-parallel).
  - Matmuls run in float32r mode (full PE rate for fp32 data).
  - Host scatter-adds routed outputs and concatenates shared slices.

Fixed problem shapes: x [2,1024,1024], E=8 experts, H=1024, I=2048, top-2,
one shared expert. The device program is compiled per capacity C (multiple
of 128 covering the max per-expert token count) and cached in-process.
"""
import math
from contextlib import ExitStack

import numpy as np

import concourse.bass as bass
import concourse.tile as tile
from concourse import bacc, mybir
from concourse.bass_utils import run_bass_kernel_spmd

F32 = mybir.dt.float32
F32R = mybir.dt.float32r
ActFn = mybir.ActivationFunctionType

P = 128
B, S, H = 2, 1024, 1024
T = B * S                    # 2048 tokens
E, TOPK, I = 8, 2, 2048
NCORES = 8
CS = T // NCORES             # shared-expert tokens per core (256)
HC = H // P                  # 8 H-chunks
IC = I // P                  # 16 I-chunks
HN = H // 512                # 2 output column chunks of 512

_program_cache: dict = {}
_last_in_maps: list | None = None


def _token_chunks(c):
    """Split c columns into chunks <=512, all >=256 when possible (fp32r
    matmuls below 256 free-dim run at 1/4 rate)."""
    n = max(1, math.ceil(c / 512))
    base = c // n
    rem = c - base * n
    out = []
    start = 0
    for j in range(n):
        sz = base + (1 if j < rem else 0)
        out.append((start, sz))
        start += sz
    return out


def _ffn(nc, pools, xt_sb, ct, w1t_d, w3t_d, w2t_d, out_d, wv_sb):
    """SwiGLU FFN: out[ct, H] = (silu(x@w1) * (x@w3)) @ w2, rows scaled by wv.

    xt_sb: SBUF tile [128, HC, ct] (fp32r) holding x transposed.
    w1t_d/w3t_d: DRAM [IC, 128, HC, 128] pre-tiled lhsT blocks.
    w2t_d: DRAM [HN, IC, 128, 512] pre-tiled rhs blocks.
    wv_sb: SBUF [128, ct//128] per-token scale, or None.
    """
    wpool, w2pool, actpool, tmppool, ypool, psum = pools
    chunks = _token_chunks(ct)
    mc = ct // P

    actT = actpool.tile([P, IC, ct], F32R)
    for i in range(IC):
        wt1 = wpool.tile([P, HC, P], F32R, tag="wt")
        nc.sync.dma_start(out=wt1, in_=w1t_d[i].bitcast(F32R))
        wt3 = wpool.tile([P, HC, P], F32R, tag="wt")
        nc.sync.dma_start(out=wt3, in_=w3t_d[i].bitcast(F32R))
        for (c0, cn) in chunks:
            ps1 = psum.tile([P, 512], F32, name="ps1", tag="ps")[:, :cn]
            for h in range(HC):
                nc.tensor.matmul(
                    ps1, lhsT=wt1[:, h, :], rhs=xt_sb[:, h, c0:c0 + cn],
                    start=(h == 0), stop=(h == HC - 1),
                )
            ps3 = psum.tile([P, 512], F32, name="ps3", tag="ps")[:, :cn]
            for h in range(HC):
                nc.tensor.matmul(
                    ps3, lhsT=wt3[:, h, :], rhs=xt_sb[:, h, c0:c0 + cn],
                    start=(h == 0), stop=(h == HC - 1),
                )
            sil = tmppool.tile([P, 512], F32, name="sil")[:, :cn]
            nc.scalar.activation(sil, ps1, ActFn.Silu)
            nc.vector.tensor_mul(actT[:, i, c0:c0 + cn], sil, ps3)

    for n in range(HN):
        w2ts = []
        for i in range(IC):
            w2t = w2pool.tile([P, 512], F32R, tag="w2")
            nc.sync.dma_start(out=w2t, in_=w2t_d[n, i].bitcast(F32R))
            w2ts.append(w2t)
        for m in range(mc):
            ps = psum.tile([P, 512], F32, name="ps", tag="ps")
            for i in range(IC):
                nc.tensor.matmul(
                    ps, lhsT=actT[:, i, m * P:(m + 1) * P], rhs=w2ts[i],
                    start=(i == 0), stop=(i == IC - 1),
                )
            yt = ypool.tile([P, 512], F32)
            if wv_sb is not None:
                nc.scalar.activation(yt, ps, ActFn.Copy, scale=wv_sb[:, m:m + 1])
            else:
                nc.scalar.activation(yt, ps, ActFn.Copy)
            nc.sync.dma_start(out=out_d[m * P:(m + 1) * P, n * 512:(n + 1) * 512], in_=yt)


def build_program(C, nloop=1):
    """Build + bass-compile the SPMD program for capacity C."""
    nc = bacc.Bacc("TRN2", target_bir_lowering=False, debug=False)

    xt = nc.dram_tensor("xt", [HC, P, C], F32, kind="ExternalInput").ap()
    wv = nc.dram_tensor("wv", [C], F32, kind="ExternalInput").ap()
    w1t = nc.dram_tensor("w1t", [IC, P, HC, P], F32, kind="ExternalInput").ap()
    w3t = nc.dram_tensor("w3t", [IC, P, HC, P], F32, kind="ExternalInput").ap()
    w2t = nc.dram_tensor("w2t", [HN, IC, P, 512], F32, kind="ExternalInput").ap()
    xst = nc.dram_tensor("xst", [HC, P, CS], F32, kind="ExternalInput").ap()
    sw1t = nc.dram_tensor("sw1t", [IC, P, HC, P], F32, kind="ExternalInput").ap()
    sw3t = nc.dram_tensor("sw3t", [IC, P, HC, P], F32, kind="ExternalInput").ap()
    sw2t = nc.dram_tensor("sw2t", [HN, IC, P, 512], F32, kind="ExternalInput").ap()
    y = nc.dram_tensor("y", [C, H], F32, kind="ExternalOutput").ap()
    ys = nc.dram_tensor("ys", [CS, H], F32, kind="ExternalOutput").ap()

    with tile.TileContext(nc) as tc:
        with ExitStack() as ctx:
            const = ctx.enter_context(tc.tile_pool(name="const", bufs=1))
            wpool = ctx.enter_context(tc.tile_pool(name="wpool", bufs=4))
            w2pool = ctx.enter_context(tc.tile_pool(name="w2pool", bufs=IC + 2))
            actpool = ctx.enter_context(tc.tile_pool(name="actpool", bufs=1))
            tmppool = ctx.enter_context(tc.tile_pool(name="tmppool", bufs=3))
            ypool = ctx.enter_context(tc.tile_pool(name="ypool", bufs=3))
            psum = ctx.enter_context(tc.tile_pool(name="psum", bufs=6, space="PSUM"))
            pools = (wpool, w2pool, actpool, tmppool, ypool, psum)

            xt_sb = const.tile([P, HC, C], F32R)
            for h in range(HC):
                nc.sync.dma_start(out=xt_sb[:, h, :], in_=xt[h].bitcast(F32R))
            xst_sb = const.tile([P, HC, CS], F32R)
            for h in range(HC):
                nc.sync.dma_start(out=xst_sb[:, h, :], in_=xst[h].bitcast(F32R))
            wv_sb = const.tile([P, C // P], F32)
            nc.sync.dma_start(out=wv_sb, in_=wv.rearrange("(m p) -> p m", p=P))

            def body():
                _ffn(nc, pools, xt_sb, C, w1t, w3t, w2t, y, wv_sb)
                _ffn(nc, pools, xst_sb, CS, sw1t, sw3t, sw2t, ys, None)

            if nloop == 1:
                body()
            else:
                with tc.For_i(0, nloop, 1):
                    body()
    nc.compile()
    return nc


def _route(xf, gate_w):
    """Replicate the reference routing in numpy fp32."""
    logits = xf @ gate_w.T                      # [T, E]
    m = logits.max(axis=1, keepdims=True)
    p = np.exp(logits - m)
    scores = p / p.sum(axis=1, keepdims=True)
    order = np.argsort(-scores, axis=1, kind="stable")[:, :TOPK]   # [T, 2]
    w_top = np.take_along_axis(scores, order, axis=1)
    w_top = w_top / (w_top.sum(axis=1, keepdims=True) + np.float32(1e-20))
    return order.astype(np.int64), w_top.astype(np.float32)


def _pretile_lhs(w):       # [H, I] -> [IC, 128, HC, 128]
    return np.ascontiguousarray(
        w.reshape(HC, P, IC, P).transpose(2, 1, 0, 3))


def _pretile_rhs(w):       # [I, H] -> [HN, IC, 128, 512]
    return np.ascontiguousarray(
        w.reshape(IC, P, HN, 512).transpose(2, 0, 1, 3))


def _pretile_x(xT):        # [H, c] -> [HC, 128, c]
    return np.ascontiguousarray(xT).reshape(HC, P, -1)


def kernel(x, gate_w, w1, w2, w3, sw1, sw2, sw3):
    x = np.asarray(x, dtype=np.float32)
    xf = x.reshape(T, H)
    order, w_top = _route(xf, np.asarray(gate_w, dtype=np.float32))

    idxs, wts = [], []
    for e in range(E):
        m0 = order[:, 0] == e
        m1 = order[:, 1] == e
        idx = np.concatenate([np.nonzero(m0)[0], np.nonzero(m1)[0]])
        wt = np.concatenate([w_top[m0, 0], w_top[m1, 1]])
        idxs.append(idx)
        wts.append(wt.astype(np.float32))

    maxn = max(len(i) for i in idxs)
    C = max(P, ((maxn + P - 1) // P) * P)

    key = C
    if key not in _program_cache:
        _program_cache[key] = build_program(C)
    nc = _program_cache[key]

    sw1t = _pretile_lhs(np.asarray(sw1, dtype=np.float32)[0])
    sw3t = _pretile_lhs(np.asarray(sw3, dtype=np.float32)[0])
    sw2t = _pretile_rhs(np.asarray(sw2, dtype=np.float32)[0])

    w1 = np.asarray(w1, dtype=np.float32)
    w2 = np.asarray(w2, dtype=np.float32)
    w3 = np.asarray(w3, dtype=np.float32)

    in_maps = []
    for c in range(NCORES):
        idx, wt = idxs[c], wts[c]
        n = len(idx)
        xte = np.zeros((H, C), dtype=np.float32)
        xte[:, :n] = xf[idx].T
        wve = np.zeros((C,), dtype=np.float32)
        wve[:n] = wt
        in_maps.append({
            "xt": _pretile_x(xte),
            "wv": wve,
            "w1t": _pretile_lhs(w1[c]),
            "w3t": _pretile_lhs(w3[c]),
            "w2t": _pretile_rhs(w2[c]),
            "xst": _pretile_x(xf[c * CS:(c + 1) * CS].T),
            "sw1t": sw1t,
            "sw3t": sw3t,
            "sw2t": sw2t,
        })

    global _last_in_maps
    _last_in_maps = in_maps
    res = run_bass_kernel_spmd(nc, in_maps, core_ids=list(range(NCORES)))

    out = np.empty((T, H), dtype=np.float32)
    for c in range(NCORES):
        out[c * CS:(c + 1) * CS] = res.results[c]["ys"]
    for c in range(NCORES):
        idx = idxs[c]
        if len(idx):
            out[idx] += res.results[c]["y"][:len(idx)]
    return out.reshape(B, S, H)


# revision 6
# speedup vs baseline: 18.1881x; 18.1881x over previous
"""Trainium2 Bass kernel for ChronosMOEFeedForward (8-expert top-2 MoE + shared expert).

Strategy (expert-parallel over 8 NeuronCores):
  - Host computes the (tiny) gate: softmax(x @ gate_w.T), top-2, normalized
    combine weights; dispatches each token to its 2 experts.
  - Core e runs a SwiGLU FFN over the tokens routed to expert e (gathered,
    transposed, padded to capacity C), scaling rows by the combine weight
    during PSUM eviction. Each core also processes a 256-token slice of the
    shared expert (data-parallel).
  - Matmuls run in float32r mode (full PE rate for fp32 data).
  - w1/w3 are concatenated host-side into one pre-tiled tensor so the weight
    stream is sequential in DRAM (interleaved streams from distant buffers
    measured 3x slower); outputs use block layouts for contiguous stores.
  - Host scatter-adds routed outputs and concatenates shared slices.

Fixed problem shapes: x [2,1024,1024], E=8 experts, H=1024, I=2048, top-2,
one shared expert. The device program is compiled per capacity C (multiple
of 128 covering the max per-expert token count) and cached in-process.
"""
import math
from contextlib import ExitStack

import numpy as np

import concourse.bass as bass
import concourse.tile as tile
from concourse import bacc, mybir
from concourse.bass_utils import run_bass_kernel_spmd

F32 = mybir.dt.float32
F32R = mybir.dt.float32r
ActFn = mybir.ActivationFunctionType

P = 128
B, S, H = 2, 1024, 1024
T = B * S                    # 2048 tokens
E, TOPK, I = 8, 2, 2048
NCORES = 8
CS = T // NCORES             # shared-expert tokens per core (256)
HC = H // P                  # 8 H-chunks
IC = I // P                  # 16 I-chunks
HN = H // 512                # 2 output column chunks of 512

_program_cache: dict = {}
_last_in_maps: list | None = None


def _token_chunks(c):
    """Split c columns into chunks <=512, all >=256 when possible (fp32r
    matmuls below 256 free-dim run at 1/4 rate)."""
    n = max(1, math.ceil(c / 512))
    base = c // n
    rem = c - base * n
    out = []
    start = 0
    for j in range(n):
        sz = base + (1 if j < rem else 0)
        out.append((start, sz))
        start += sz
    return out


def _ffn(nc, pools, xt_sb, ct, w13t_d, w2t_d, yb_d, wv_sb):
    """SwiGLU FFN over ct tokens; output written as [HN, ct//128, 128, 512]
    blocks, rows scaled by wv when given.

    xt_sb: SBUF tile [128, HC, ct] (fp32r) holding x transposed.
    w13t_d: DRAM [IC, 2, 128, HC, 128] pre-tiled lhsT blocks (w1|w3 fused).
    w2t_d: DRAM [HN, IC, 128, 512] pre-tiled rhs blocks.
    """
    wpool, w2pool, actpool, tmppool, ypool, psum = pools
    chunks = _token_chunks(ct)
    mc = ct // P

    actT = actpool.tile([P, IC, ct], F32R, name="actT", tag="actT")
    for i in range(IC):
        wt13 = wpool.tile([P, 2, HC, P], F32R, tag="wt", name="wt13")
        nc.sync.dma_start(
            out=wt13, in_=w13t_d[i].rearrange("w p h c -> p w h c").bitcast(F32R))
        for (c0, cn) in chunks:
            ps1 = psum.tile([P, 512], F32, name="ps1", tag="ps")[:, :cn]
            for h in range(HC):
                nc.tensor.matmul(
                    ps1, lhsT=wt13[:, 0, h, :], rhs=xt_sb[:, h, c0:c0 + cn],
                    start=(h == 0), stop=(h == HC - 1),
                )
            ps3 = psum.tile([P, 512], F32, name="ps3", tag="ps")[:, :cn]
            for h in range(HC):
                nc.tensor.matmul(
                    ps3, lhsT=wt13[:, 1, h, :], rhs=xt_sb[:, h, c0:c0 + cn],
                    start=(h == 0), stop=(h == HC - 1),
                )
            sil = tmppool.tile([P, 512], F32, name="sil")[:, :cn]
            nc.scalar.activation(sil, ps1, ActFn.Silu)
            nc.vector.tensor_mul(actT[:, i, c0:c0 + cn], sil, ps3)

    for n in range(HN):
        w2ts = []
        for i in range(IC):
            w2t = w2pool.tile([P, 512], F32R, tag="w2", name="w2t_sb")
            nc.sync.dma_start(out=w2t, in_=w2t_d[n, i].bitcast(F32R))
            w2ts.append(w2t)
        for m in range(mc):
            ps = psum.tile([P, 512], F32, name="ps", tag="ps")
            for i in range(IC):
                nc.tensor.matmul(
                    ps, lhsT=actT[:, i, m * P:(m + 1) * P], rhs=w2ts[i],
                    start=(i == 0), stop=(i == IC - 1),
                )
            yt = ypool.tile([P, 512], F32, name="yt")
            if wv_sb is not None:
                nc.scalar.activation(yt, ps, ActFn.Copy, scale=wv_sb[:, m:m + 1])
            else:
                nc.scalar.activation(yt, ps, ActFn.Copy)
            nc.sync.dma_start(out=yb_d[n, m], in_=yt)


def build_program(C, nloop=1):
    """Build + bass-compile the SPMD program for capacity C."""
    nc = bacc.Bacc("TRN2", target_bir_lowering=False, debug=False)

    MC = C // P
    xt = nc.dram_tensor("xt", [HC, P, C], F32, kind="ExternalInput").ap()
    wv = nc.dram_tensor("wv", [C], F32, kind="ExternalInput").ap()
    w13t = nc.dram_tensor("w13t", [IC, 2, P, HC, P], F32, kind="ExternalInput").ap()
    w2t = nc.dram_tensor("w2t", [HN, IC, P, 512], F32, kind="ExternalInput").ap()
    xst = nc.dram_tensor("xst", [HC, P, CS], F32, kind="ExternalInput").ap()
    sw13t = nc.dram_tensor("sw13t", [IC, 2, P, HC, P], F32, kind="ExternalInput").ap()
    sw2t = nc.dram_tensor("sw2t", [HN, IC, P, 512], F32, kind="ExternalInput").ap()
    yb = nc.dram_tensor("yb", [HN, MC, P, 512], F32, kind="ExternalOutput").ap()
    ysb = nc.dram_tensor("ysb", [HN, CS // P, P, 512], F32, kind="ExternalOutput").ap()

    with tile.TileContext(nc) as tc:
        with ExitStack() as ctx:
            const = ctx.enter_context(tc.tile_pool(name="const", bufs=1))
            wpool = ctx.enter_context(tc.tile_pool(name="wpool", bufs=3))
            w2pool = ctx.enter_context(tc.tile_pool(name="w2pool", bufs=IC + 2))
            actpool = ctx.enter_context(tc.tile_pool(name="actpool", bufs=1))
            tmppool = ctx.enter_context(tc.tile_pool(name="tmppool", bufs=3))
            ypool = ctx.enter_context(tc.tile_pool(name="ypool", bufs=3))
            psum = ctx.enter_context(tc.tile_pool(name="psum", bufs=6, space="PSUM"))
            pools = (wpool, w2pool, actpool, tmppool, ypool, psum)

            xt_sb = const.tile([P, HC, C], F32R)
            for h in range(HC):
                nc.sync.dma_start(out=xt_sb[:, h, :], in_=xt[h].bitcast(F32R))
            xst_sb = const.tile([P, HC, CS], F32R)
            for h in range(HC):
                nc.sync.dma_start(out=xst_sb[:, h, :], in_=xst[h].bitcast(F32R))
            wv_sb = const.tile([P, C // P], F32)
            nc.sync.dma_start(out=wv_sb, in_=wv.rearrange("(m p) -> p m", p=P))

            def body():
                _ffn(nc, pools, xt_sb, C, w13t, w2t, yb, wv_sb)
                _ffn(nc, pools, xst_sb, CS, sw13t, sw2t, ysb, None)

            if nloop == 1:
                body()
            else:
                with tc.For_i(0, nloop, 1):
                    body()
    nc.compile()
    return nc


def _route(xf, gate_w):
    """Replicate the reference routing in numpy fp32."""
    logits = xf @ gate_w.T                      # [T, E]
    m = logits.max(axis=1, keepdims=True)
    p = np.exp(logits - m)
    scores = p / p.sum(axis=1, keepdims=True)
    order = np.argsort(-scores, axis=1, kind="stable")[:, :TOPK]   # [T, 2]
    w_top = np.take_along_axis(scores, order, axis=1)
    w_top = w_top / (w_top.sum(axis=1, keepdims=True) + np.float32(1e-20))
    return order.astype(np.int64), w_top.astype(np.float32)


def _pretile_lhs13(w1, w3):   # 2 x [H, I] -> [IC, 2, 128, HC, 128]
    out = np.empty((IC, 2, P, HC, P), dtype=np.float32)
    out[:, 0] = w1.reshape(HC, P, IC, P).transpose(2, 1, 0, 3)
    out[:, 1] = w3.reshape(HC, P, IC, P).transpose(2, 1, 0, 3)
    return out


def _pretile_rhs(w):          # [I, H] -> [HN, IC, 128, 512]
    return np.ascontiguousarray(
        w.reshape(IC, P, HN, 512).transpose(2, 0, 1, 3))


def _unblock(yb, ct):         # [HN, ct//128, 128, 512] -> [ct, H]
    return yb.transpose(1, 2, 0, 3).reshape(ct, H)


def kernel(x, gate_w, w1, w2, w3, sw1, sw2, sw3):
    x = np.asarray(x, dtype=np.float32)
    xf = x.reshape(T, H)
    order, w_top = _route(xf, np.asarray(gate_w, dtype=np.float32))

    idxs, wts = [], []
    for e in range(E):
        m0 = order[:, 0] == e
        m1 = order[:, 1] == e
        idx = np.concatenate([np.nonzero(m0)[0], np.nonzero(m1)[0]])
        wt = np.concatenate([w_top[m0, 0], w_top[m1, 1]])
        idxs.append(idx)
        wts.append(wt.astype(np.float32))

    maxn = max(len(i) for i in idxs)
    C = max(P, ((maxn + P - 1) // P) * P)

    if C not in _program_cache:
        _program_cache[C] = build_program(C)
    nc = _program_cache[C]

    sw13t = _pretile_lhs13(np.asarray(sw1, dtype=np.float32)[0],
                           np.asarray(sw3, dtype=np.float32)[0])
    sw2t = _pretile_rhs(np.asarray(sw2, dtype=np.float32)[0])

    w1 = np.asarray(w1, dtype=np.float32)
    w2 = np.asarray(w2, dtype=np.float32)
    w3 = np.asarray(w3, dtype=np.float32)

    in_maps = []
    for c in range(NCORES):
        idx, wt = idxs[c], wts[c]
        n = len(idx)
        xte = np.zeros((H, C), dtype=np.float32)
        xte[:, :n] = xf[idx].T
        wve = np.zeros((C,), dtype=np.float32)
        wve[:n] = wt
        in_maps.append({
            "xt": np.ascontiguousarray(xte).reshape(HC, P, C),
            "wv": wve,
            "w13t": _pretile_lhs13(w1[c], w3[c]),
            "w2t": _pretile_rhs(w2[c]),
            "xst": np.ascontiguousarray(xf[c * CS:(c + 1) * CS].T).reshape(HC, P, CS),
            "sw13t": sw13t,
            "sw2t": sw2t,
        })

    global _last_in_maps
    _last_in_maps = in_maps
    res = run_bass_kernel_spmd(nc, in_maps, core_ids=list(range(NCORES)))

    out = np.empty((T, H), dtype=np.float32)
    for c in range(NCORES):
        out[c * CS:(c + 1) * CS] = _unblock(res.results[c]["ysb"], CS)
    for c in range(NCORES):
        idx = idxs[c]
        if len(idx):
            out[idx] += _unblock(res.results[c]["yb"], C)[:len(idx)]
    return out.reshape(B, S, H)
